# revision 1
# baseline (speedup 1.0000x reference)
"""Transformer encoder layer (B=4, S=2048, D=512, F=2048) on 8 trn2 NeuronCores.

Sharding: data-parallel over batch (4 batches) x 2-way split over query
positions -> 8 cores, no collectives. Each core computes full K/V for its
batch (duplicated across the pair of cores sharing a batch; ~5% extra FLOPs)
and 1024 queries end-to-end.

Per-core layout strategy:
  - q/k/v are pre-transposed on the host to feature-major [D, tokens] so the
    d-contraction projections need no on-device transposes.
  - QT/KT are produced feature-major [e, tokens]; scores^T [k, q] comes out of
    the PE directly, exp() runs on ScalarE into P^T [k, q] (bf16), and
    attention is matmul(lhsT=P^T slice, rhs=V), which lands token-major [q, e]
    with no transpose of the 2M-element probability matrix.
  - Softmax skips max-subtraction (scores ~ N(0,1) by construction) and gets
    denominators from matmul(P^T, ones[128,2]) accumulated over k tiles;
    since numerator and denominator use the same bf16 P^T, normalization is
    self-consistent and the bf16 cost is only the V-rounding (~2e-4).
  - bv is folded into the residual x on the host (softmax rows sum to 1).
  - LayerNorm applies (x-mu)*rstd with one ScalarE Identity activation using
    per-partition scale/bias APs.
  - Projection/FFN matmuls run in float32r (full PE rate, ~1e-4 error);
    attention probs/V run in bf16.
  - scores chunk 1 is emitted between the K and V projections so the vT DMA
    (which reuses the kT SBUF zone) overlaps scores compute.
"""

import sys

for _p in ("/opt/trn_rl_repo",):
    if _p not in sys.path:
        sys.path.append(_p)

import numpy as np
from contextlib import ExitStack

import concourse.bacc as bacc
import concourse.tile as tile
from concourse import mybir
from concourse.bass_utils import run_bass_kernel_spmd

P = 128
B, S, D, F = 4, 2048, 512, 2048
SQ = S // 2          # queries per core
NCORES = 8
EPS = 1e-5
F32 = mybir.dt.float32
F32R = mybir.dt.float32r
BF16 = mybir.dt.bfloat16
AF = mybir.ActivationFunctionType

DT = D // P          # 4  d tiles
ET = D // P          # 4  e tiles
NKT = S // P         # 16 key-token tiles
KC = S // 512        # 4  key chunks of 512
QC = SQ // 512       # 2  query chunks of 512
QS = SQ // P         # 8  query subtiles of 128
FT = F // P          # 16 f tiles

INV_SQRT_D = 1.0 / float(np.sqrt(D))

_PROGRAM_CACHE = {}


def _build(need_gb1: bool, need_b2: bool, need_gb2: bool):
    nc = bacc.Bacc()

    qT_d = nc.declare_dram_parameter("qT", [D, SQ], F32R, isOutput=False)
    kT_d = nc.declare_dram_parameter("kT", [D, S], F32R, isOutput=False)
    vT_d = nc.declare_dram_parameter("vTb", [D, S], BF16, isOutput=False)
    x_d = nc.declare_dram_parameter("x", [SQ, D], F32, isOutput=False)
    wqkv_d = nc.declare_dram_parameter("Wqk", [D, 2 * D], F32R, isOutput=False)
    wvb_d = nc.declare_dram_parameter("Wvb", [D, D], BF16, isOutput=False)
    w1_d = nc.declare_dram_parameter("W1", [D, F], F32R, isOutput=False)
    w2_d = nc.declare_dram_parameter("W2", [F, D], F32R, isOutput=False)
    bq_d = nc.declare_dram_parameter("bq", [D], F32, isOutput=False)
    bk_d = nc.declare_dram_parameter("bk", [D], F32, isOutput=False)
    b1_d = nc.declare_dram_parameter("b1", [F], F32, isOutput=False)
    ident_d = nc.declare_dram_parameter("ident", [P, P], F32R, isOutput=False)
    out_d = nc.declare_dram_parameter("out", [SQ, D], F32, isOutput=True)
    if need_gb1:
        g1_d = nc.declare_dram_parameter("g1", [D], F32, isOutput=False)
        be1_d = nc.declare_dram_parameter("be1", [D], F32, isOutput=False)
    if need_b2:
        b2_d = nc.declare_dram_parameter("b2", [D], F32, isOutput=False)
    if need_gb2:
        g2_d = nc.declare_dram_parameter("g2", [D], F32, isOutput=False)
        be2_d = nc.declare_dram_parameter("be2", [D], F32, isOutput=False)

    with tile.TileContext(nc) as tc, ExitStack() as ctx:
        const = ctx.enter_context(tc.tile_pool(name="const", bufs=1))
        psum = ctx.enter_context(tc.tile_pool(name="psum", bufs=1, space="PSUM"))

        # ---- constants (small DMAs on gpsimd to keep the sync queue clear) --
        ident_sb = const.tile([P, P], F32R, name="ident_sb")
        ones_b = const.tile([P, 2], BF16, name="ones_b")
        nc.vector.memset(ones_b, 1.0)
        eps_t = const.tile([P, 1], F32, name="eps_t")
        nc.vector.memset(eps_t, EPS)
        bq_sb = const.tile([P, ET], F32, name="bq_sb")
        bk_sb = const.tile([P, ET], F32, name="bk_sb")
        b1_sb = const.tile([P, FT], F32, name="b1_sb")

        def bcast_row(src_ap, nm):
            row = const.tile([1, D], F32, name=f"{nm}_row")
            nc.gpsimd.dma_start(out=row[:], in_=src_ap[None, :])
            rowr = const.tile([1, D], F32R, name=f"{nm}_rowr")
            nc.scalar.activation(rowr[:], row[:], AF.Copy)
            onesrow = const.tile([1, P], F32, name=f"{nm}_of")
            nc.vector.memset(onesrow, 1.0)
            onesrow_r = const.tile([1, P], F32R, name=f"{nm}_or")
            nc.scalar.activation(onesrow_r[:], onesrow[:], AF.Copy)
            ps_b = psum.tile([P, D], F32, name=f"ps_{nm}", tag="tr", bufs=2)
            nc.tensor.matmul(ps_b[:], onesrow_r[:], rowr[:], start=True, stop=True)
            full = const.tile([P, D], F32, name=f"{nm}_full")
            nc.scalar.activation(full[:], ps_b[:], AF.Copy)
            return full

        # ---- long-lived h tiles ----
        hpool = ctx.enter_context(tc.tile_pool(name="hpool", bufs=1))
        h = [hpool.tile([P, D], F32R, name=f"h{i}") for i in range(QS)]

        def layer_norm_emit(pool, y, out_tile, g_full, be_full, out_is_f32r, key):
            """y: [128, D] fp32 SBUF tile -> out_tile = LN(y) (*g +be)."""
            stats = pool.tile([P, 6], F32, name=f"st_{key}", tag="st", bufs=4)
            nc.vector.bn_stats(out=stats[:], in_=y[:])
            mv = pool.tile([P, 2], F32, name=f"mv_{key}", tag="mv", bufs=4)
            nc.vector.bn_aggr(out=mv[:], in_=stats[:])
            std = pool.tile([P, 1], F32, name=f"sd_{key}", tag="sd", bufs=4)
            nc.scalar.activation(std[:], mv[:, 1:2], AF.Sqrt, bias=eps_t[:])
            rstd = pool.tile([P, 1], F32, name=f"rs_{key}", tag="rs", bufs=4)
            nc.vector.reciprocal(rstd[:], std[:])
            nmr = pool.tile([P, 1], F32, name=f"nm_{key}", tag="nm", bufs=4)
            nc.vector.tensor_mul(nmr[:], mv[:, 0:1], rstd[:])
            nc.scalar.mul(nmr[:], nmr[:], -1.0)
            if g_full is None:
                nc.scalar.activation(
                    out_tile[:], y[:], AF.Identity, bias=nmr[:], scale=rstd[:]
                )
            else:
                t = pool.tile([P, D], F32, name=f"lt_{key}", tag="lt", bufs=2)
                nc.scalar.activation(t[:], y[:], AF.Identity, bias=nmr[:], scale=rstd[:])
                nc.vector.tensor_mul(t[:], t[:], g_full[:])
                if out_is_f32r:
                    t2 = pool.tile([P, D], F32, name=f"lu_{key}", tag="lu", bufs=2)
                    nc.vector.tensor_add(t2[:], t[:], be_full[:])
                    nc.scalar.activation(out_tile[:], t2[:], AF.Copy)
                else:
                    nc.vector.tensor_add(out_tile[:], t[:], be_full[:])

        # ---- pools for the projection/attention era (stack order matters) --
        vpool_cm = tc.tile_pool(name="vpool", bufs=1)
        vpool = vpool_cm.__enter__()
        V = [vpool.tile([P, D], BF16, name=f"V{kt}") for kt in range(NKT)]

        qkp_cm = tc.tile_pool(name="qkp", bufs=1)
        qkp = qkp_cm.__enter__()
        QT = [[qkp.tile([P, 512], F32R, name=f"QT{e}_{qc}") for qc in range(QC)] for e in range(ET)]
        KTl = [[qkp.tile([P, 512], F32R, name=f"KT{e}_{kc}") for kc in range(KC)] for e in range(ET)]

        projw_cm = tc.tile_pool(name="projw", bufs=1)
        projw = projw_cm.__enter__()
        wqkv_sb = [projw.tile([P, 2 * D], F32R, name=f"wqkv{d}") for d in range(DT)]
        wvb_sb = [projw.tile([P, D], BF16, name=f"wvb{d}") for d in range(DT)]
        vTb_sb = [projw.tile([P, S], BF16, name=f"vTb{d}") for d in range(DT)]

        inpa_cm = tc.tile_pool(name="inpa", bufs=1)
        inpa = inpa_cm.__enter__()
        qT_sb = [inpa.tile([P, SQ], F32R, name=f"qT{d}") for d in range(DT)]
        kT_sb = [inpa.tile([P, S], F32R, name=f"kT{d}") for d in range(DT)]

        # DMA issue order on sync: vTb/Wvb (V-proj runs first; only 2.5MB),
        # then qT/Wqk for QT, then kT. Halved transfers spread across queues.
        for d in range(DT):
            nc.sync.dma_start(out=vTb_sb[d][:, :S//2], in_=vT_d[d * P : (d + 1) * P, :S//2])
            nc.sync.dma_start(out=vTb_sb[d][:, S//2:], in_=vT_d[d * P : (d + 1) * P, S//2:])
            nc.sync.dma_start(out=wvb_sb[d][:], in_=wvb_d[d * P : (d + 1) * P, :])
        for d in range(DT):
            nc.sync.dma_start(out=qT_sb[d][:, :SQ//2], in_=qT_d[d * P : (d + 1) * P, :SQ//2])
            nc.sync.dma_start(out=qT_sb[d][:, SQ//2:], in_=qT_d[d * P : (d + 1) * P, SQ//2:])
        for d in range(DT):
            nc.sync.dma_start(out=wqkv_sb[d][:, :D], in_=wqkv_d[d * P : (d + 1) * P, :D])
            nc.sync.dma_start(out=wqkv_sb[d][:, D:], in_=wqkv_d[d * P : (d + 1) * P, D:])
        for d in range(DT):
            nc.sync.dma_start(out=kT_sb[d][:, :S//2], in_=kT_d[d * P : (d + 1) * P, :S//2])
            nc.sync.dma_start(out=kT_sb[d][:, S//2:], in_=kT_d[d * P : (d + 1) * P, S//2:])
        # small constant loads ride the gpsimd queue
        nc.gpsimd.dma_start(out=ident_sb[:], in_=ident_d[:, :])
        nc.gpsimd.dma_start(out=bq_sb[:], in_=bq_d.rearrange("(a p) -> p a", p=P))
        nc.gpsimd.dma_start(out=bk_sb[:], in_=bk_d.rearrange("(a p) -> p a", p=P))
        nc.gpsimd.dma_start(out=b1_sb[:], in_=b1_d.rearrange("(a p) -> p a", p=P))

        g1_full = be1_full = b2_full = g2_full = be2_full = None
        if need_gb1:
            g1_full = bcast_row(g1_d, "g1")
            be1_full = bcast_row(be1_d, "be1")
        if need_b2:
            b2_full = bcast_row(b2_d, "b2")
        if need_gb2:
            g2_full = bcast_row(g2_d, "g2")
            be2_full = bcast_row(be2_d, "be2")

        def wq(d):
            return wqkv_sb[d][:, 0:D]

        def wk(d):
            return wqkv_sb[d][:, D : 2 * D]

        # ---- V projection first (smallest DMA prerequisite), d-outer over
        # quads of 4 psum groups so the first matmul needs only vTb[0]'s first
        # half + Wvb[0] (~640KB) instead of all of vTb/Wvb.
        for ktq in range(NKT // 4):
            pss = [
                psum.tile([P, D], F32, name=f"ps_v{ktq}_{j}", tag="mm", bufs=4)
                for j in range(4)
            ]
            for d in range(DT):
                for j in range(4):
                    kt = ktq * 4 + j
                    nc.tensor.matmul(
                        pss[j][:],
                        vTb_sb[d][:, kt * P : (kt + 1) * P],
                        wvb_sb[d][:],
                        start=(d == 0),
                        stop=(d == DT - 1),
                    )
            for j in range(4):
                nc.scalar.activation(V[ktq * 4 + j][:], pss[j][:], AF.Copy)

        # ---- Q and K projections ----
        for e in range(ET):
            for qc in range(QC):
                ps = psum.tile([P, 512], F32, name=f"ps_q{e}_{qc}", tag="mm", bufs=4)
                for d in range(DT):
                    nc.tensor.matmul(
                        ps[:],
                        wq(d)[:, e * P : (e + 1) * P],
                        qT_sb[d][:, qc * 512 : (qc + 1) * 512],
                        start=(d == 0),
                        stop=(d == DT - 1),
                    )
                nc.scalar.activation(
                    QT[e][qc][:], ps[:], AF.Identity, bias=bq_sb[:, e : e + 1]
                )
        for e in range(ET):
            for kc in range(KC):
                ps = psum.tile([P, 512], F32, name=f"ps_k{e}_{kc}", tag="mm", bufs=4)
                for d in range(DT):
                    nc.tensor.matmul(
                        ps[:],
                        wk(d)[:, e * P : (e + 1) * P],
                        kT_sb[d][:, kc * 512 : (kc + 1) * 512],
                        start=(d == 0),
                        stop=(d == DT - 1),
                    )
                nc.scalar.activation(
                    KTl[e][kc][:], ps[:], AF.Identity, bias=bk_sb[:, e : e + 1]
                )

        inpa_cm.__exit__(None, None, None)

        # attention-era pools reuse the qT/kT zone; vT's DMA overlaps scores c1
        xp_cm = tc.tile_pool(name="xp", bufs=1)
        xp = xp_cm.__enter__()
        x_sb = [xp.tile([P, 4 * D], F32, name=f"x{g}") for g in range(2)]
        for g in range(2):
            nc.gpsimd.dma_start(
                out=x_sb[g].rearrange("p (j e) -> p j e", j=4),
                in_=x_d[g * 512 : (g + 1) * 512, :].rearrange("(j p) e -> p j e", p=P),
            )

        yp_cm = tc.tile_pool(name="yp", bufs=1)
        yp = yp_cm.__enter__()

        lnpa_cm = tc.tile_pool(name="lnpa", bufs=1)
        lnpa = lnpa_cm.__enter__()

        ptp_cm = tc.tile_pool(name="ptp", bufs=1)
        ptp = ptp_cm.__enter__()
        PT = [
            [ptp.tile([P, 512], BF16, name=f"PT{kt}_c{qc}", tag=f"PT{kt}", bufs=1) for kt in range(NKT)]
            for qc in range(QC)
        ]

        def scores_chunk(qc):
            for kt in range(NKT):
                ps = psum.tile([P, 512], F32, name=f"ps_s{kt}_{qc}", tag="mm", bufs=4)
                for e in range(ET):
                    nc.tensor.matmul(
                        ps[:],
                        KTl[e][kt // 4][:, (kt % 4) * P : (kt % 4 + 1) * P],
                        QT[e][qc][:],
                        start=(e == 0),
                        stop=(e == ET - 1),
                    )
                nc.scalar.activation(PT[qc][kt][:], ps[:], AF.Exp, scale=INV_SQRT_D)

        y_tiles = []

        def attn_chunk(qc):
            for q4 in range(4):
                qs = qc * 4 + q4
                ps_a = psum.tile([P, D], F32, name=f"ps_a{qs}", tag="mm", bufs=4)
                for kt in range(NKT):
                    nc.tensor.matmul(
                        ps_a[:],
                        PT[qc][kt][:, q4 * P : (q4 + 1) * P],
                        V[kt][:],
                        start=(kt == 0),
                        stop=(kt == NKT - 1),
                    )
                ps_dn = psum.tile([P, 2], F32, name=f"ps_dn{qs}", tag="dn", bufs=2)
                for kt in range(NKT):
                    nc.tensor.matmul(
                        ps_dn[:],
                        PT[qc][kt][:, q4 * P : (q4 + 1) * P],
                        ones_b[:],
                        start=(kt == 0),
                        stop=(kt == NKT - 1),
                    )
                recip = lnpa.tile([P, 1], F32, name=f"rc{qs}", tag="rc", bufs=4)
                nc.vector.reciprocal(recip[:], ps_dn[:, 0:1])
                y = yp.tile([P, D], F32, name=f"y{qs}")
                nc.vector.tensor_scalar_mul(y[:], ps_a[:], recip[:])
                nc.vector.tensor_add(
                    y[:], y[:], x_sb[qs // 4][:, (qs % 4) * D : (qs % 4 + 1) * D]
                )
                y_tiles.append((qs, y))

        scores_chunk(0)
        attn_chunk(0)
        scores_chunk(1)
        attn_chunk(1)

        # LayerNorm 1 (deferred so ScalarE switches exp -> sqrt tables once)
        for qs, y in y_tiles:
            layer_norm_emit(lnpa, y, h[qs], g1_full, be1_full, True, f"h{qs}")

        ptp_cm.__exit__(None, None, None)
        lnpa_cm.__exit__(None, None, None)
        yp_cm.__exit__(None, None, None)
        xp_cm.__exit__(None, None, None)
        projw_cm.__exit__(None, None, None)
        qkp_cm.__exit__(None, None, None)
        vpool_cm.__exit__(None, None, None)

        # ---- FFN era ----
        w1p_cm = tc.tile_pool(name="w1p", bufs=1)
        w1p = w1p_cm.__enter__()
        w1_sb = [w1p.tile([P, F], F32R, name=f"w1_{d}") for d in range(DT)]
        for d in range(DT):
            nc.sync.dma_start(out=w1_sb[d][:], in_=w1_d[d * P : (d + 1) * P, :])

        w2p_cm = tc.tile_pool(name="w2p", bufs=1)
        w2p = w2p_cm.__enter__()
        w2_sb = [w2p.tile([P, 4 * D], F32R, name=f"w2_{g}") for g in range(4)]
        for g in range(4):
            nc.gpsimd.dma_start(
                out=w2_sb[g].rearrange("p (j e) -> p j e", j=4),
                in_=w2_d[g * 512 : (g + 1) * 512, :].rearrange("(j p) e -> p j e", p=P),
            )

        ffp_cm = tc.tile_pool(name="ffp", bufs=1)
        ffp = ffp_cm.__enter__()
        hT = [[ffp.tile([P, 512], F32R, name=f"hT{d}_{qc}") for qc in range(QC)] for d in range(DT)]
        for qs in range(QS):
            qc, q4 = qs // 4, qs % 4
            for d in range(DT):
                ps_t = psum.tile(
                    [P, P], F32R, name=f"ps_t{qs}_{d}",
                    tag=("tr" if (qs * DT + d) % 2 == 0 else "dn"), bufs=2,
                )
                nc.tensor.transpose(ps_t[:], h[qs][:, d * P : (d + 1) * P], ident_sb[:])
                nc.scalar.activation(
                    hT[d][qc][:, q4 * P : (q4 + 1) * P], ps_t.bitcast(F32)[:], AF.Copy
                )

        lnpb_cm = tc.tile_pool(name="lnpb", bufs=1)
        lnpb = lnpb_cm.__enter__()

        # FFN1/FFN2 per query chunk; fT slots are reused across chunks
        fT = [
            [ffp.tile([P, 512], F32R, name=f"fT{f}_c{qc}", tag=f"fT{f}", bufs=1) for f in range(FT)]
            for qc in range(QC)
        ]
        for qc in range(QC):
            for f in range(FT):
                ps = psum.tile([P, 512], F32, name=f"ps_f{f}_{qc}", tag="mm", bufs=4)
                for d in range(DT):
                    nc.tensor.matmul(
                        ps[:],
                        w1_sb[d][:, f * P : (f + 1) * P],
                        hT[d][qc][:],
                        start=(d == 0),
                        stop=(d == DT - 1),
                    )
                nc.scalar.activation(
                    fT[qc][f][:], ps[:], AF.Relu, bias=b1_sb[:, f : f + 1]
                )
            for q4 in range(4):
                qs = qc * 4 + q4
                ps = psum.tile([P, D], F32, name=f"ps_o{qs}", tag="mm", bufs=4)
                for f in range(FT):
                    nc.tensor.matmul(
                        ps[:],
                        fT[qc][f][:, q4 * P : (q4 + 1) * P],
                        w2_sb[f // 4][:, (f % 4) * D : (f % 4 + 1) * D],
                        start=(f == 0),
                        stop=(f == FT - 1),
                    )
                y2 = lnpb.tile([P, D], F32, name=f"y2_{qs}", tag="y2", bufs=3)
                nc.vector.tensor_add(y2[:], ps[:], h[qs].bitcast(F32)[:])
                if b2_full is not None:
                    nc.vector.tensor_add(y2[:], y2[:], b2_full[:])
                out_t = lnpb.tile([P, D], F32, name=f"ot{qs}", tag="ot", bufs=3)
                layer_norm_emit(lnpb, y2, out_t, g2_full, be2_full, False, f"o{qs}")
                nc.sync.dma_start(out=out_d[qs * P : (qs + 1) * P, :], in_=out_t[:])

        lnpb_cm.__exit__(None, None, None)
        ffp_cm.__exit__(None, None, None)
        w2p_cm.__exit__(None, None, None)
        w1p_cm.__exit__(None, None, None)

    nc.compile()
    return nc


def _get_program(need_gb1, need_b2, need_gb2):
    key = (need_gb1, need_b2, need_gb2)
    if key not in _PROGRAM_CACHE:
        _PROGRAM_CACHE[key] = _build(*key)
    return _PROGRAM_CACHE[key]


def kernel(
    q, k, v, x, Wq, bq, Wk, bk, Wv, bv, g1, be1, W1, b1, W2, b2, g2, be2, _trace=False
):
    q = np.asarray(q, dtype=np.float32)
    k = np.asarray(k, dtype=np.float32)
    v = np.asarray(v, dtype=np.float32)
    x = np.asarray(x, dtype=np.float32)

    need_gb1 = bool(np.any(np.asarray(g1) != 1.0) or np.any(np.asarray(be1) != 0.0))
    need_b2 = bool(np.any(np.asarray(b2) != 0.0))
    need_gb2 = bool(np.any(np.asarray(g2) != 1.0) or np.any(np.asarray(be2) != 0.0))

    nc = _get_program(need_gb1, need_b2, need_gb2)

    np_bf16 = mybir.dt.np(BF16)
    wqk = np.concatenate(
        [np.asarray(Wq, dtype=np.float32), np.asarray(Wk, dtype=np.float32)], axis=1
    )
    shared = {
        "Wqk": np.ascontiguousarray(wqk),
        "Wvb": np.ascontiguousarray(Wv, dtype=np.float32).astype(np_bf16),
        "W1": np.ascontiguousarray(W1, dtype=np.float32),
        "W2": np.ascontiguousarray(W2, dtype=np.float32),
        "bq": np.ascontiguousarray(bq, dtype=np.float32),
        "bk": np.ascontiguousarray(bk, dtype=np.float32),
        "b1": np.ascontiguousarray(b1, dtype=np.float32),
        "ident": np.eye(P, dtype=np.float32),
    }
    if need_gb1:
        shared["g1"] = np.ascontiguousarray(g1, dtype=np.float32)
        shared["be1"] = np.ascontiguousarray(be1, dtype=np.float32)
    if need_b2:
        shared["b2"] = np.ascontiguousarray(b2, dtype=np.float32)
    if need_gb2:
        shared["g2"] = np.ascontiguousarray(g2, dtype=np.float32)
        shared["be2"] = np.ascontiguousarray(be2, dtype=np.float32)

    bv32 = np.asarray(bv, dtype=np.float32)
    in_maps = []
    for c in range(NCORES):
        b, half = c // 2, c % 2
        sl = slice(half * SQ, (half + 1) * SQ)
        in_maps.append(
            {
                "qT": np.ascontiguousarray(q[b, sl].T),
                "kT": np.ascontiguousarray(k[b].T),
                "vTb": np.ascontiguousarray(v[b].T).astype(np_bf16),
                "x": np.ascontiguousarray(x[b, sl]) + bv32[None, :],
                **shared,
            }
        )

    res = run_bass_kernel_spmd(nc, in_maps, list(range(NCORES)), trace=_trace)

    out = np.empty((B, S, D), dtype=np.float32)
    for c in range(NCORES):
        b, half = c // 2, c % 2
        out[b, half * SQ : (half + 1) * SQ] = res.results[c]["out"]
    if _trace:
        return out, res
    return out



# revision 10
# speedup vs baseline: 1.2995x; 1.2995x over previous
"""Transformer encoder layer (B=4, S=2048, D=512, F=2048) on 8 trn2 NeuronCores.

Sharding: data-parallel over batch (4 batches) x 2-way split over query
positions -> 8 cores, no collectives. Each core computes full K/V for its
batch and 1024 queries end-to-end.

Per-core strategy (fp8 DoubleRow attention + bf16 FFN):
  - q/k/v inputs, Wq/Wk/Wv (pre-scaled x32 on host; 1/32 folded into the
    psum evacuation) are fp8e4m3. All projection/attention matmuls use
    MatmulPerfMode.DoubleRow: lhsT/rhs carry [128, 2, N] APs contracting
    256-deep per instruction at 2x bf16 rate.
  - Layouts keep contraction on partitions with the 2-tile pair on a middle
    free dim: qT8/kT8/vT8 [128, dt, tok], W8 [128, dt, 512], QT8 [128, et, q],
    KT8 [128, et, k], V8 [128, kt, 512], PT8 [128, kt, q].
  - Softmax skips max-subtraction; exp carries bias -ln(4) so fp8 PT values
    stay below e4m3 max even for ~6-sigma scores (the 1/4 cancels between
    numerator and the matmul-ones denominator, which uses the same fp8 PT,
    so normalization is self-consistent).
  - bv is folded into the residual x on the host (softmax rows sum to 1).
  - FFN runs in bf16 (fp8 would put ~3% RMS on the final output; bf16 is
    negligible): h transposed via PE bf16 transposes, W1/W2 bf16.
  - Non-PE work is spread across engines so the PE stays the bottleneck:
    ACT does exp/relu/LN-applies, Pool (gpsimd) does fp8 psum evacuations
    and bf16 h copies, DVE does LN stats, softmax normalize, residual adds.
"""

import sys

for _p in ("/opt/trn_rl_repo",):
    if _p not in sys.path:
        sys.path.append(_p)

import numpy as np
from contextlib import ExitStack

import concourse.bacc as bacc
import concourse.tile as tile
from concourse import mybir
from concourse.bass_utils import run_bass_kernel_spmd

P = 128
B, S, D, F = 4, 2048, 512, 2048
SQ = S // 2          # queries per core
NCORES = 8
EPS = 1e-5
F32 = mybir.dt.float32
BF16 = mybir.dt.bfloat16
FP8 = mybir.dt.float8e4
AF = mybir.ActivationFunctionType
ALU = mybir.AluOpType
DR = mybir.MatmulPerfMode.DoubleRow

DT = D // P          # 4  d tiles
ET = D // P          # 4  e tiles
NKT = S // P         # 16 key-token tiles
KC = S // 512        # 4  key chunks of 512
QC = SQ // 512       # 2  query chunks of 512
QS = SQ // P         # 8  query subtiles of 128
FT = F // P          # 16 f tiles

INV_SQRT_D = 1.0 / float(np.sqrt(D))
WSCALE = 32.0        # host pre-scales fp8 weights (avoids e4m3 subnormals)
EXP_BIAS = -float(np.log(4.0))   # keeps exp() under e4m3 max

_PROGRAM_CACHE = {}
DEBUG_TAPS = False


def _build(need_gb1: bool, need_b2: bool, need_gb2: bool):
    nc = bacc.Bacc()

    qT_d = nc.declare_dram_parameter("qT8", [P, DT * SQ], FP8, isOutput=False)
    kT_d = nc.declare_dram_parameter("kT8", [P, DT * S], FP8, isOutput=False)
    vT_d = nc.declare_dram_parameter("vT8", [P, DT * S], FP8, isOutput=False)
    w8_d = nc.declare_dram_parameter("w8", [P, 12 * 512], FP8, isOutput=False)
    x_d = nc.declare_dram_parameter("x", [P, QS * D], F32, isOutput=False)
    w1_d = nc.declare_dram_parameter("w1", [P, DT * F], BF16, isOutput=False)
    w2_d = nc.declare_dram_parameter("w2", [P, FT * D], BF16, isOutput=False)
    bqk_d = nc.declare_dram_parameter("bqk", [P, 2 * ET], F32, isOutput=False)
    b1_d = nc.declare_dram_parameter("b1", [P, FT], F32, isOutput=False)
    ident_d = nc.declare_dram_parameter("ident", [P, P], BF16, isOutput=False)
    out_d = nc.declare_dram_parameter("out", [SQ, D], F32, isOutput=True)
    if need_gb1:
        g1_d = nc.declare_dram_parameter("g1", [D], F32, isOutput=False)
        be1_d = nc.declare_dram_parameter("be1", [D], F32, isOutput=False)
    if need_b2:
        b2_d = nc.declare_dram_parameter("b2", [D], F32, isOutput=False)
    if need_gb2:
        g2_d = nc.declare_dram_parameter("g2", [D], F32, isOutput=False)
        be2_d = nc.declare_dram_parameter("be2", [D], F32, isOutput=False)

    with tile.TileContext(nc) as tc, ExitStack() as ctx:
        const = ctx.enter_context(tc.tile_pool(name="const", bufs=1))
        psum = ctx.enter_context(tc.tile_pool(name="psum", bufs=1, space="PSUM"))

        ident_sb = const.tile([P, P], BF16, name="ident_sb")
        ones8 = const.tile([P, 4], FP8, name="ones8")
        nc.vector.memset(ones8, 1.0)
        eps_t = const.tile([P, 1], F32, name="eps_t")
        nc.vector.memset(eps_t, EPS)
        expb_t = const.tile([P, 1], F32, name="expb_t")
        nc.vector.memset(expb_t, EXP_BIAS)
        bqk_sb = const.tile([P, 2 * ET], F32, name="bqk_sb")
        b1_sb = const.tile([P, FT], F32, name="b1_sb")

        def bcast_row(src_ap, nm):
            row = const.tile([1, D], F32, name=f"{nm}_row")
            nc.gpsimd.dma_start(out=row[:], in_=src_ap[None, :])
            rowb = const.tile([1, D], BF16, name=f"{nm}_rowb")
            nc.scalar.activation(rowb[:], row[:], AF.Copy)
            onesrow = const.tile([1, P], F32, name=f"{nm}_of")
            nc.vector.memset(onesrow, 1.0)
            onesrow_b = const.tile([1, P], BF16, name=f"{nm}_or")
            nc.scalar.activation(onesrow_b[:], onesrow[:], AF.Copy)
            ps_b = psum.tile([P, D], F32, name=f"ps_{nm}", tag="tr", bufs=2)
            nc.tensor.matmul(ps_b[:], onesrow_b[:], rowb[:], start=True, stop=True)
            full = const.tile([P, D], F32, name=f"{nm}_full")
            nc.scalar.activation(full[:], ps_b[:], AF.Copy)
            return full

        # ---- long-lived h tiles ----
        hpool = ctx.enter_context(tc.tile_pool(name="hpool", bufs=1))
        h_res = [hpool.tile([P, D], F32, name=f"h{i}") for i in range(QS)]
        h_bf = [hpool.tile([P, D], BF16, name=f"hb{i}") for i in range(QS)]

        def layer_norm_emit(pool, y, out_tile, g_full, be_full, key):
            """y: [128, D] fp32 tile -> out_tile = LN(y) (*g +be)."""
            stats = pool.tile([P, 6], F32, name=f"st_{key}", tag="st", bufs=4)
            nc.vector.bn_stats(out=stats[:], in_=y[:])
            mv = pool.tile([P, 2], F32, name=f"mv_{key}", tag="mv", bufs=4)
            nc.vector.bn_aggr(out=mv[:], in_=stats[:])
            std = pool.tile([P, 1], F32, name=f"sd_{key}", tag="sd", bufs=4)
            nc.scalar.activation(std[:], mv[:, 1:2], AF.Sqrt, bias=eps_t[:])
            rstd = pool.tile([P, 1], F32, name=f"rs_{key}", tag="rs", bufs=4)
            nc.vector.reciprocal(rstd[:], std[:])
            nmr = pool.tile([P, 1], F32, name=f"nm_{key}", tag="nm", bufs=4)
            nc.vector.tensor_mul(nmr[:], mv[:, 0:1], rstd[:])
            nc.scalar.mul(nmr[:], nmr[:], -1.0)
            if g_full is None:
                nc.scalar.activation(
                    out_tile[:], y[:], AF.Identity, bias=nmr[:], scale=rstd[:]
                )
            else:
                t = pool.tile([P, D], F32, name=f"lt_{key}", tag="lt", bufs=2)
                nc.scalar.activation(t[:], y[:], AF.Identity, bias=nmr[:], scale=rstd[:])
                nc.vector.tensor_mul(t[:], t[:], g_full[:])
                nc.vector.tensor_add(out_tile[:], t[:], be_full[:])

        # ---- FFN weights (persistent; DMAs issued last in queue order) ----
        w12 = ctx.enter_context(tc.tile_pool(name="w12", bufs=1))
        w1_sb = w12.tile([P, DT * F], BF16, name="w1_sb")
        w2_sb = w12.tile([P, FT * D], BF16, name="w2_sb")

        # ---- attention-era pools ----
        era_a = tc.tile_pool(name="era_a", bufs=1)
        ea = era_a.__enter__()
        w8_sb = ea.tile([P, 12 * 512], FP8, name="w8_sb")
        qT_sb = ea.tile([P, DT * SQ], FP8, name="qT_sb")
        kT_sb = ea.tile([P, DT * S], FP8, name="kT_sb")
        vT_sb = ea.tile([P, DT * S], FP8, name="vT_sb")
        QT8 = ea.tile([P, ET * SQ], FP8, name="QT8")
        KT8 = ea.tile([P, ET * S], FP8, name="KT8")
        V8 = ea.tile([P, NKT * D], FP8, name="V8")
        PT8 = [ea.tile([P, NKT * 512], FP8, name=f"PT8_{qc}") for qc in range(QC)]
        x_sb = ea.tile([P, QS * D], F32, name="x_sb")
        y_tiles = [ea.tile([P, D], F32, name=f"y{qs}") for qs in range(QS)]

        # 3-D views: [partition, tile-pair axis, free]
        w8r = w8_sb.rearrange("p (a e) -> p a e", a=12)
        qTr = qT_sb.rearrange("p (a q) -> p a q", a=DT)
        kTr = kT_sb.rearrange("p (a k) -> p a k", a=DT)
        vTr = vT_sb.rearrange("p (a k) -> p a k", a=DT)
        QTr = QT8.rearrange("p (a q) -> p a q", a=ET)
        KTr = KT8.rearrange("p (a k) -> p a k", a=ET)
        V8r = V8.rearrange("p (a e) -> p a e", a=NKT)
        PTr = [PT8[qc].rearrange("p (a q) -> p a q", a=NKT) for qc in range(QC)]
        onesr = ones8.rearrange("p (a t) -> p a t", a=2)
        xr = x_sb.rearrange("p (a e) -> p a e", a=QS)

        # ---- DMA issue order on the sync queue tracks compute order ----
        # w8 layout: wq tiles 0:4, wk 4:8, wv 8:12
        nc.sync.dma_start(out=w8_sb[:, : 6 * 512], in_=w8_d[:, : 6 * 512])
        nc.sync.dma_start(out=w8_sb[:, 6 * 512 :], in_=w8_d[:, 6 * 512 :])
        nc.sync.dma_start(out=qT_sb[:, : DT * SQ // 2], in_=qT_d[:, : DT * SQ // 2])
        nc.sync.dma_start(out=qT_sb[:, DT * SQ // 2 :], in_=qT_d[:, DT * SQ // 2 :])
        for half in range(2):
            sl = slice(half * DT * S // 2, (half + 1) * DT * S // 2)
            nc.sync.dma_start(out=kT_sb[:, sl], in_=kT_d[:, sl])
        for half in range(2):
            sl = slice(half * DT * S // 2, (half + 1) * DT * S // 2)
            nc.sync.dma_start(out=vT_sb[:, sl], in_=vT_d[:, sl])
        for half in range(2):
            sl = slice(half * QS * D // 2, (half + 1) * QS * D // 2)
            nc.sync.dma_start(out=x_sb[:, sl], in_=x_d[:, sl])
        for half in range(2):
            sl = slice(half * DT * F // 2, (half + 1) * DT * F // 2)
            nc.sync.dma_start(out=w1_sb[:, sl], in_=w1_d[:, sl])
        for half in range(2):
            sl = slice(half * FT * D // 2, (half + 1) * FT * D // 2)
            nc.sync.dma_start(out=w2_sb[:, sl], in_=w2_d[:, sl])
        # small constant loads ride the gpsimd queue
        nc.gpsimd.dma_start(out=ident_sb[:], in_=ident_d[:, :])
        nc.gpsimd.dma_start(out=bqk_sb[:], in_=bqk_d[:, :])
        nc.gpsimd.dma_start(out=b1_sb[:], in_=b1_d[:, :])

        g1_full = be1_full = b2_full = g2_full = be2_full = None
        if need_gb1:
            g1_full = bcast_row(g1_d, "g1")
            be1_full = bcast_row(be1_d, "be1")
        if need_b2:
            b2_full = bcast_row(b2_d, "b2")
        if need_gb2:
            g2_full = bcast_row(g2_d, "g2")
            be2_full = bcast_row(be2_d, "be2")

        IW = 1.0 / WSCALE

        # ---- Q projection: out [128e, 512q] = sum_d Wq[d,e]^T qT[d,q] ----
        for e in range(ET):
            for qc in range(QC):
                ps = psum.tile([P, 512], F32, name=f"ps_q{e}_{qc}", tag="mm", bufs=4)
                for i in range(DT // 2):
                    nc.tensor.matmul(
                        ps[:],
                        w8r[:, 2 * i : 2 * i + 2, e * P : (e + 1) * P],
                        qTr[:, 2 * i : 2 * i + 2, qc * 512 : (qc + 1) * 512],
                        start=(i == 0),
                        stop=(i == DT // 2 - 1),
                        perf_mode=DR,
                    )
                nc.vector.tensor_scalar(
                    QTr[:, e, qc * 512 : (qc + 1) * 512],
                    ps[:],
                    IW,
                    bqk_sb[:, e : e + 1],
                    ALU.mult,
                    ALU.add,
                )
        # ---- K projection ----
        for e in range(ET):
            for kc in range(KC):
                ps = psum.tile([P, 512], F32, name=f"ps_k{e}_{kc}", tag="mm", bufs=4)
                for i in range(DT // 2):
                    nc.tensor.matmul(
                        ps[:],
                        w8r[:, 4 + 2 * i : 4 + 2 * i + 2, e * P : (e + 1) * P],
                        kTr[:, 2 * i : 2 * i + 2, kc * 512 : (kc + 1) * 512],
                        start=(i == 0),
                        stop=(i == DT // 2 - 1),
                        perf_mode=DR,
                    )
                nc.vector.tensor_scalar(
                    KTr[:, e, kc * 512 : (kc + 1) * 512],
                    ps[:],
                    IW,
                    bqk_sb[:, ET + e : ET + e + 1],
                    ALU.mult,
                    ALU.add,
                )
        # ---- V projection: out [128tok, 512e] = sum_d vT[d,tok]^T Wv[d,e] ----
        for kt in range(NKT):
            ps = psum.tile([P, 512], F32, name=f"ps_v{kt}", tag="mm", bufs=4)
            for i in range(DT // 2):
                nc.tensor.matmul(
                    ps[:],
                    vTr[:, 2 * i : 2 * i + 2, kt * P : (kt + 1) * P],
                    w8r[:, 8 + 2 * i : 8 + 2 * i + 2, :],
                    start=(i == 0),
                    stop=(i == DT // 2 - 1),
                    perf_mode=DR,
                )
            nc.scalar.mul(V8r[:, kt, :], ps[:], IW)

        # ---- scores + attention, per 512-query chunk ----
        def scores_chunk(qc):
            for kt in range(NKT):
                ps = psum.tile([P, 512], F32, name=f"ps_s{kt}_{qc}", tag="mm", bufs=4)
                for i in range(ET // 2):
                    nc.tensor.matmul(
                        ps[:],
                        KTr[:, 2 * i : 2 * i + 2, kt * P : (kt + 1) * P],
                        QTr[:, 2 * i : 2 * i + 2, qc * 512 : (qc + 1) * 512],
                        start=(i == 0),
                        stop=(i == ET // 2 - 1),
                        perf_mode=DR,
                    )
                nc.scalar.activation(
                    PTr[qc][:, kt, :], ps[:], AF.Exp, scale=INV_SQRT_D, bias=expb_t[:]
                )

        ln_inputs = []

        def attn_chunk(qc):
            for q4 in range(4):
                qs = qc * 4 + q4
                ps_a = psum.tile([P, D], F32, name=f"ps_a{qs}", tag="mm", bufs=4)
                for i in range(NKT // 2):
                    nc.tensor.matmul(
                        ps_a[:],
                        PTr[qc][:, 2 * i : 2 * i + 2, q4 * P : (q4 + 1) * P],
                        V8r[:, 2 * i : 2 * i + 2, :],
                        start=(i == 0),
                        stop=(i == NKT // 2 - 1),
                        perf_mode=DR,
                    )
                ps_dn = psum.tile([P, 2], F32, name=f"ps_dn{qs}", tag="dn", bufs=2)
                for i in range(NKT // 2):
                    nc.tensor.matmul(
                        ps_dn[:],
                        PTr[qc][:, 2 * i : 2 * i + 2, q4 * P : (q4 + 1) * P],
                        onesr[:, :, :],
                        start=(i == 0),
                        stop=(i == NKT // 2 - 1),
                        perf_mode=DR,
                    )
                recip = ea.tile([P, 1], F32, name=f"rc{qs}", tag="rc", bufs=4)
                nc.vector.reciprocal(recip[:], ps_dn[:, 0:1])
                y = y_tiles[qs]
                nc.vector.tensor_scalar_mul(y[:], ps_a[:], recip[:])
                nc.vector.tensor_add(y[:], y[:], xr[:, qs, :])
                ln_inputs.append((qs, y))

        scores_chunk(0)
        attn_chunk(0)
        scores_chunk(1)
        attn_chunk(1)

        if DEBUG_TAPS:
            dbg = {
                "dbg_QT8": (QT8, FP8, [P, ET * SQ]),
                "dbg_KT8": (KT8, FP8, [P, ET * S]),
                "dbg_V8": (V8, FP8, [P, NKT * D]),
                "dbg_PT0": (PT8[0], FP8, [P, NKT * 512]),
                "dbg_y0": (y_tiles[0], F32, [P, D]),
            }
            for nm, (t, dt_, shp) in dbg.items():
                d = nc.declare_dram_parameter(nm, shp, dt_, isOutput=True)
                nc.sync.dma_start(out=d[:, :], in_=t[:])

        # LayerNorm 1 (deferred so ScalarE switches exp -> sqrt tables once)
        lnpa_cm = tc.tile_pool(name="lnpa", bufs=1)
        lnpa = lnpa_cm.__enter__()
        for qs, y in ln_inputs:
            layer_norm_emit(lnpa, y, h_res[qs], g1_full, be1_full, f"h{qs}")
            nc.gpsimd.tensor_copy(h_bf[qs][:], h_res[qs][:])
        lnpa_cm.__exit__(None, None, None)
        era_a.__exit__(None, None, None)

        # ---- FFN era ----
        ffp_cm = tc.tile_pool(name="ffp", bufs=1)
        ffp = ffp_cm.__enter__()
        hT = [ffp.tile([P, DT * 512], BF16, name=f"hT{qc}") for qc in range(QC)]
        hTr = [hT[qc].rearrange("p (a q) -> p a q", a=DT) for qc in range(QC)]
        fT = [ffp.tile([P, FT * 512], BF16, name=f"fT{qc}") for qc in range(QC)]
        fTr = [fT[qc].rearrange("p (a q) -> p a q", a=FT) for qc in range(QC)]
        w1r = w1_sb.rearrange("p (a f) -> p a f", a=DT)
        w2r = w2_sb.rearrange("p (a e) -> p a e", a=FT)

        for qs in range(QS):
            qc, q4 = qs // 4, qs % 4
            for d in range(DT):
                ps_t = psum.tile(
                    [P, P], BF16, name=f"ps_t{qs}_{d}",
                    tag=("tr" if (qs * DT + d) % 2 == 0 else "dn"), bufs=2,
                )
                nc.tensor.transpose(ps_t[:], h_bf[qs][:, d * P : (d + 1) * P], ident_sb[:])
                nc.vector.tensor_copy(hTr[qc][:, d, q4 * P : (q4 + 1) * P], ps_t[:])

        lnpb_cm = tc.tile_pool(name="lnpb", bufs=1)
        lnpb = lnpb_cm.__enter__()

        for qc in range(QC):
            for f in range(FT):
                ps = psum.tile([P, 512], F32, name=f"ps_f{f}_{qc}", tag="mm", bufs=4)
                for d in range(DT):
                    nc.tensor.matmul(
                        ps[:],
                        w1r[:, d, f * P : (f + 1) * P],
                        hTr[qc][:, d, :],
                        start=(d == 0),
                        stop=(d == DT - 1),
                    )
                nc.scalar.activation(
                    fTr[qc][:, f, :], ps[:], AF.Relu, bias=b1_sb[:, f : f + 1]
                )
            for q4 in range(4):
                qs = qc * 4 + q4
                ps = psum.tile([P, D], F32, name=f"ps_o{qs}", tag="mm", bufs=4)
                for f in range(FT):
                    nc.tensor.matmul(
                        ps[:],
                        fTr[qc][:, f, q4 * P : (q4 + 1) * P],
                        w2r[:, f, :],
                        start=(f == 0),
                        stop=(f == FT - 1),
                    )
                y2 = lnpb.tile([P, D], F32, name=f"y2_{qs}", tag="y2", bufs=3)
                nc.vector.tensor_add(y2[:], ps[:], h_res[qs][:])
                if b2_full is not None:
                    nc.vector.tensor_add(y2[:], y2[:], b2_full[:])
                out_t = lnpb.tile([P, D], F32, name=f"ot{qs}", tag="ot", bufs=3)
                layer_norm_emit(lnpb, y2, out_t, g2_full, be2_full, f"o{qs}")
                nc.sync.dma_start(out=out_d[qs * P : (qs + 1) * P, :], in_=out_t[:])

        lnpb_cm.__exit__(None, None, None)
        ffp_cm.__exit__(None, None, None)

    nc.compile()
    return nc


def _get_program(need_gb1, need_b2, need_gb2):
    key = (need_gb1, need_b2, need_gb2)
    if key not in _PROGRAM_CACHE:
        _PROGRAM_CACHE[key] = _build(*key)
    return _PROGRAM_CACHE[key]


def _to_tiled(a, ntiles):
    """[R, C] with R = ntiles*128 -> [128, ntiles*C] laid out [p, tile, c]."""
    r, c = a.shape
    return np.ascontiguousarray(
        a.reshape(ntiles, P, c).transpose(1, 0, 2).reshape(P, ntiles * c)
    )


def kernel(
    q, k, v, x, Wq, bq, Wk, bk, Wv, bv, g1, be1, W1, b1, W2, b2, g2, be2, _trace=False
):
    q = np.asarray(q, dtype=np.float32)
    k = np.asarray(k, dtype=np.float32)
    v = np.asarray(v, dtype=np.float32)
    x = np.asarray(x, dtype=np.float32)

    need_gb1 = bool(np.any(np.asarray(g1) != 1.0) or np.any(np.asarray(be1) != 0.0))
    need_b2 = bool(np.any(np.asarray(b2) != 0.0))
    need_gb2 = bool(np.any(np.asarray(g2) != 1.0) or np.any(np.asarray(be2) != 0.0))

    nc = _get_program(need_gb1, need_b2, need_gb2)

    np_fp8 = mybir.dt.np(FP8)
    np_bf16 = mybir.dt.np(BF16)

    def wprep(W):
        # [D, D] -> [128, dt, 512] fp8, pre-scaled
        return (
            np.asarray(W, np.float32).reshape(DT, P, D).transpose(1, 0, 2)
            * WSCALE
        ).astype(np_fp8).reshape(P, DT * D)

    w8 = np.concatenate([wprep(Wq), wprep(Wk), wprep(Wv)], axis=1)
    w1h = (
        np.asarray(W1, np.float32).reshape(DT, P, F).transpose(1, 0, 2)
    ).astype(np_bf16).reshape(P, DT * F)
    w2h = (
        np.asarray(W2, np.float32).reshape(FT, P, D).transpose(1, 0, 2)
    ).astype(np_bf16).reshape(P, FT * D)
    bqk = np.concatenate(
        [
            np.asarray(bq, np.float32).reshape(ET, P).T,
            np.asarray(bk, np.float32).reshape(ET, P).T,
        ],
        axis=1,
    )
    b1h = np.ascontiguousarray(np.asarray(b1, np.float32).reshape(FT, P).T)

    shared = {
        "w8": np.ascontiguousarray(w8),
        "w1": np.ascontiguousarray(w1h),
        "w2": np.ascontiguousarray(w2h),
        "bqk": np.ascontiguousarray(bqk),
        "b1": b1h,
        "ident": np.eye(P, dtype=np.float32).astype(np_bf16),
    }
    if need_gb1:
        shared["g1"] = np.ascontiguousarray(g1, dtype=np.float32)
        shared["be1"] = np.ascontiguousarray(be1, dtype=np.float32)
    if need_b2:
        shared["b2"] = np.ascontiguousarray(b2, dtype=np.float32)
    if need_gb2:
        shared["g2"] = np.ascontiguousarray(g2, dtype=np.float32)
        shared["be2"] = np.ascontiguousarray(be2, dtype=np.float32)

    bv32 = np.asarray(bv, dtype=np.float32)
    in_maps = []
    for c in range(NCORES):
        b, half = c // 2, c % 2
        sl = slice(half * SQ, (half + 1) * SQ)
        # feature-major [D, tokens] -> [128, dt, tokens] fp8
        qT = _to_tiled(np.ascontiguousarray(q[b, sl].T), DT).astype(np_fp8)
        kT = _to_tiled(np.ascontiguousarray(k[b].T), DT).astype(np_fp8)
        vT = _to_tiled(np.ascontiguousarray(v[b].T), DT).astype(np_fp8)
        xh = _to_tiled(x[b, sl] + bv32[None, :], QS)
        in_maps.append(
            {"qT8": qT, "kT8": kT, "vT8": vT, "x": xh, **shared}
        )

    res = run_bass_kernel_spmd(nc, in_maps, list(range(NCORES)), trace=_trace)

    out = np.empty((B, S, D), dtype=np.float32)
    for c in range(NCORES):
        b, half = c // 2, c % 2
        out[b, half * SQ : (half + 1) * SQ] = res.results[c]["out"]
    if _trace:
        return out, res
    return out
